# revision 27
# baseline (speedup 1.0000x reference)
"""Trainium2 Bass kernel for nn_E_GCL (EGNN graph conv layer).

Contract: kernel(**inputs) takes the FULL unsharded inputs (as produced by
setup_inputs) and returns the full outputs (h_out [B,N,D], x_out [B,N,3]).

Strategy (hardcoded for B=8, N=256, D=H=128, 8 NeuronCores):
- Data-parallel over batch: core b processes sample b.
- Host-side: per sample, permute nodes so mask==1 nodes come first, and
  restrict the O(N^2) edge computation to the first NP=153 nodes (actual
  active counts are <= 147; guarded by a numpy fallback). Masked nodes
  inside [0,NP) are neutralized by adding -1e4 to the edge-MLP layer-1
  pre-activation (SiLU(-1e4)=0 exactly, and zero biases make all
  downstream masked quantities exactly 0, matching the reference's cm
  masking).
- Device: all big matmuls in bf16 with fp32 PSUM accumulation.
  dist_sq is decomposed as r_i + r_j - 2 x_i.x_j; the r_i/r_j rank-1
  terms are folded into per-node A''/B'' vectors, so the per-edge
  layer-1 pre-activation is  A''_i + B''_j + we1_c (x) (-2 x_i.x_j),
  assembled directly in PSUM by one fused K=10 matmul per bank (indicator
  rows for A'' + a Db row for the dist channel), with B'' added by one
  broadcast DVE op per group.
- Edge loop: 17 groups of 9 i-nodes; per group three [128, 1377]
  activation passes (SiLU m1, SiLU m2, SiLU c1). W = tanh(c1 @ wc2) is
  computed as [1, GW] row strips (stationary wc2), tanh'd on ACT, then
  DMA-repacked to row-major W; coord sums are DVE multiply-reduces
  against partition-broadcast x columns.
- Node path (all 256 nodes): agg_m from per-group reduces; node MLP,
  residual in fp32, LayerNorm via PE transpose + bn_stats.
"""

import os
import sys

import numpy as np

if "/opt/trn_rl_repo" not in sys.path:
    sys.path.insert(0, "/opt/trn_rl_repo")

import ml_dtypes

BF = ml_dtypes.bfloat16

# problem constants
B, N, D, H = 8, 256, 128, 128
NP = 153          # padded compacted node count for the edge computation
GS = 9            # i-nodes per group
NG = NP // GS     # 17 groups
BW = 3 * NP       # per-PSUM-bank used width (459 fp32 <= 512)
R2 = NP - 128     # rows in the second partition block (25)
GW = GS * NP      # per-group width (1296)
LN_EPS = 1e-5
MASK_NEG = -1.0e4
CSCALE = 0.1 / N  # tanh post-scale folded with the 1/N coord normalizer


def _silu(v):
    return v * (1.0 / (1.0 + np.exp(-v)))


def _numpy_reference(h, x, mask, we1, be1, we2, be2, wn1, bn1, wn2, bn2,
                     wc1, bc1, wc2, gamma, beta):
    """Exact fp32 fallback (only used if the fast-path guards fail)."""
    m = mask.astype(h.dtype)
    cm = m[:, :, None] * m[:, None, :]
    hi = np.broadcast_to(h[:, :, None, :], (B, N, N, D))
    hj = np.broadcast_to(h[:, None, :, :], (B, N, N, D))
    rel_x = x[:, :, None, :] - x[:, None, :, :]
    dist_sq = np.sum(rel_x * rel_x, axis=-1, keepdims=True) + 1e-8
    edge_input = np.concatenate([hi, hj, np.broadcast_to(dist_sq, (B, N, N, 1))], axis=-1)
    m_ij = _silu(_silu(edge_input @ we1 + be1) @ we2 + be2)
    m_ij = m_ij * cm[..., None]
    w_ij = np.tanh(_silu(m_ij @ wc1 + bc1) @ wc2) * 0.1
    w_ij = w_ij * cm[..., None]
    coord_diff = np.sum(rel_x * w_ij, axis=2) / N
    x_out = x + coord_diff
    agg_m = np.sum(m_ij, axis=2)
    h_new = _silu(np.concatenate([h, agg_m], axis=-1) @ wn1 + bn1) @ wn2 + bn2
    h2 = h + h_new
    mu = h2.mean(axis=-1, keepdims=True)
    var = ((h2 - mu) ** 2).mean(axis=-1, keepdims=True)
    h_out = (h2 - mu) / np.sqrt(var + LN_EPS) * gamma + beta
    return h_out.astype(np.float32), x_out.astype(np.float32)


def build_nc(use_silu=True):
    """Build the single-core Bass program (same NEFF runs SPMD on 8 cores)."""
    import concourse.bacc as bacc
    import concourse.tile as tile
    import concourse.mybir as mybir

    F32 = mybir.dt.float32
    BF16 = mybir.dt.bfloat16
    AF = mybir.ActivationFunctionType
    OP = mybir.AluOpType

    nc = bacc.Bacc("TRN2", target_bir_lowering=False, debug=False)

    # ---- DRAM tensors: shared weights (same values on every core) ----
    din = {}

    def dram_in(name, shape, dt):
        din[name] = nc.dram_tensor(name, list(shape), dt, kind="ExternalInput")
        return din[name]

    dram_in("we1a", (D, H), BF16)       # we1[:D]
    dram_in("we1b", (D, H), BF16)       # we1[D:2D]
    dram_in("cw2", (2, H), BF16)        # [we1_c ; ones]
    dram_in("we2", (H, H), BF16)
    dram_in("wc1", (H, H), BF16)
    dram_in("wc2", (H, 1), BF16)
    dram_in("wn1a", (D, H), BF16)       # wn1[:D]
    dram_in("wn1b", (H, H), BF16)       # wn1[D:]
    dram_in("wn2", (H, D), BF16)
    dram_in("be1", (H, 1), F32)
    dram_in("be2", (H, 1), F32)
    dram_in("bc1", (H, 1), F32)
    dram_in("bn1", (H, 1), F32)
    dram_in("bn2", (D, 1), F32)
    dram_in("gamma", (1, D), F32)
    dram_in("beta", (1, D), F32)
    dram_in("identf", (128, 128), F32)
    dram_in("ind9", (GS, GW), BF16)     # IND9[k, (k', j)] = (k == k')
    # ---- per-core (per-sample, permuted) tensors ----
    dram_in("hT32", (D, N), F32)        # h permuted, transposed, fp32
    dram_in("hTb", (D, N), BF16)
    dram_in("xt32", (3, N), F32)        # x permuted, transposed
    dram_in("xtb", (3, N), BF16)
    dram_in("x32", (128, 2, 3), F32)    # x permuted, natural, [p, blk, 3]
    dram_in("mnegb", (1, N), BF16)      # -1e4 * (1 - mask), permuted

    ho_d = nc.dram_tensor("h_out", [N, D], F32, kind="ExternalOutput")
    xo_d = nc.dram_tensor("x_out", [N, 3], F32, kind="ExternalOutput")

    SILU = AF.Silu if use_silu else AF.Sigmoid

    with tile.TileContext(nc) as tc:
        with (
            tc.tile_pool(name="sb", bufs=1) as sb,          # persistent sbuf
            tc.tile_pool(name="lp", bufs=3) as lp,          # loop sbuf tiles
            tc.tile_pool(name="ps", bufs=2, space="PSUM") as ps,
            tc.tile_pool(name="nps", bufs=1, space="PSUM") as nps,
        ):
            # ---------------- load everything into SBUF ----------------
            t = {}
            for name, shape, dt in [
                ("we1a", (D, H), BF16), ("we1b", (D, H), BF16),
                ("cw2", (2, H), BF16), ("we2", (H, H), BF16),
                ("wc1", (H, H), BF16), ("wc2", (H, 1), BF16),
                ("wn1a", (D, H), BF16), ("wn1b", (H, H), BF16),
                ("wn2", (H, D), BF16),
                ("be1", (H, 1), F32), ("be2", (H, 1), F32),
                ("bc1", (H, 1), F32), ("bn1", (H, 1), F32),
                ("bn2", (D, 1), F32),
                ("identf", (128, 128), F32), ("ind9", (GS, GW), BF16),
                ("hT32", (D, N), F32), ("hTb", (D, N), BF16),
                ("xt32", (3, N), F32), ("xtb", (3, N), BF16),
                ("x32", (128, 2, 3), F32),
            ]:
                t[name] = sb.tile(list(shape), dt, tag=f"t_{name}",
                                  name=f"t_{name}")
                eng = nc.gpsimd if name[0] == "w" or name[0] == "b" else nc.sync
                eng.dma_start(t[name][:], din[name].ap())

            # gamma/beta broadcast to [128, D] once
            gamB = sb.tile([128, D], F32, tag="gamB")
            betB = sb.tile([128, D], F32, tag="betB")
            nc.scalar.dma_start(gamB[:], din["gamma"].ap().to_broadcast((128, D)))
            nc.scalar.dma_start(betB[:], din["beta"].ap().to_broadcast((128, D)))

            # ---------------- per-sample prep ----------------
            # r = |x|^2 row:  Square(xt) then ones3^T @ xt2
            xt2 = sb.tile([3, N], BF16, tag="xt2")
            nc.scalar.activation(out=xt2[:], in_=t["xt32"][:], func=AF.Square)
            ones3 = sb.tile([3, 1], BF16, tag="ones3")
            nc.vector.memset(ones3[:], 1.0)
            r_ps = ps.tile([1, 512], F32, tag="ps")
            nc.tensor.matmul(r_ps[0:1, 0:N], ones3[:], xt2[:], start=True, stop=True)
            # rm2 = [r ; mneg] (2, N) bf16
            rm2 = sb.tile([2, N], BF16, tag="rm2")
            nc.vector.tensor_copy(rm2[0:1, :], r_ps[0:1, 0:N])
            nc.sync.dma_start(rm2[1:2, :], din["mnegb"].ap())

            # B'' = we1b^T hT + we1c (x) r + ones (x) mneg   -> bf16 [H, NP]
            bpp_ps = ps.tile([H, 512], F32, tag="ps")
            nc.tensor.matmul(bpp_ps[:, 0:N], t["we1b"][:], t["hTb"][:],
                             start=True, stop=False)
            nc.tensor.matmul(bpp_ps[:, 0:N], t["cw2"][:], rm2[:],
                             start=False, stop=True)
            bpp = sb.tile([H, NP], BF16, tag="bpp")
            nc.vector.tensor_copy(bpp[:], bpp_ps[:, 0:NP])

            # A''^T chunk tiles: aTc[c][k, u, h] for g = 4c+u; row GS = we1_c
            # so a single K=GS+1 matmul applies A'' and the dist channel
            NCH = (NG + 3) // 4
            aTc = []
            for c in range(NCH):
                gs_in = min(4, NG - 4 * c)
                ac = sb.tile([GS + 1, gs_in, H], BF16, tag=f"aTc{c}",
                             name=f"aTc{c}")
                pa = ps.tile([GS, 4, H], F32, tag="ps", name=f"pa_{c}")
                for u in range(gs_in):
                    g = 4 * c + u
                    nc.tensor.matmul(pa[0:GS, u, :],
                                     t["hTb"][:, g * GS:(g + 1) * GS],
                                     t["we1a"][:], start=True, stop=False)
                    nc.tensor.matmul(pa[0:GS, u, :],
                                     rm2[:, g * GS:(g + 1) * GS],
                                     t["cw2"][:], start=False, stop=True)
                nc.vector.tensor_copy(ac[0:GS, :, :], pa[0:GS, 0:gs_in, :])
                nc.sync.dma_start(
                    ac[GS:GS + 1, :, :],
                    din["cw2"].ap()[0:1, :].rearrange("p (o h) -> p o h", o=1)
                        .to_broadcast((1, gs_in, H)))
                aTc.append(ac)

            # T10: rhs rows for the fused m1pre matmul.  Rows 0..GS-1 hold the
            # IND indicator pattern (replicated per group), row GS holds
            # Db[i, j] = -2 x_i . x_j flattened in (i, j) order.
            t10 = sb.tile([GS + 1, NG * GW], BF16, tag="t10")
            nc.scalar.dma_start(
                t10[0:GS, :],
                din["ind9"].ap().rearrange("k (o w) -> k o w", o=1)
                    .to_broadcast((GS, NG, GW)))
            db = sb.tile([128, 2, NP], BF16, tag="db")
            for blk, rows in ((0, 128), (1, R2)):
                gp = ps.tile([128, 512], F32, tag="ps")
                nc.tensor.matmul(gp[0:rows, 0:NP],
                                 t["xtb"][:, blk * 128: blk * 128 + rows],
                                 t["xtb"][:, 0:NP], start=True, stop=True)
                nc.vector.tensor_scalar(out=db[0:rows, blk, :], in0=gp[0:rows, 0:NP],
                                        scalar1=-2.0, scalar2=None, op0=OP.mult)
                nc.sync.dma_start(
                    t10[GS:GS + 1, blk * 128 * NP: blk * 128 * NP + rows * NP],
                    db[0:rows, blk, :])

            # AGG (sum_j m_ij) fp32 [H, N]; cols >= NP stay 0
            agg = sb.tile([H, N], F32, tag="agg")
            nc.vector.memset(agg[:], 0.0)
            aggb = sb.tile([H, NP], BF16, tag="aggb")

            # node-MLP layer-1 psum accumulates across the edge loop, split
            # into column halves so the first half's LN can overlap the loop
            n1A = nps.tile([H, 128], F32, tag="n1A")
            n1B = nps.tile([H, 128], F32, tag="n1B")
            nc.tensor.matmul(n1A[:, :], t["wn1a"][:], t["hTb"][:, 0:128],
                             start=True, stop=False)
            nc.tensor.matmul(n1B[:, :], t["wn1a"][:], t["hTb"][:, 128:N],
                             start=True, stop=False)

            # row-major W accumulators (filled by per-group DMA repacks)
            wrow0 = sb.tile([128, NP], BF16, tag="wrow0")
            wrow1 = sb.tile([R2, NP], BF16, tag="wrow1")
            # partition-broadcast x columns for the coord reduces
            xb = []
            for k in range(3):
                xbk = sb.tile([128, NP], BF16, tag=f"xb{k}", name=f"xb{k}")
                nc.scalar.dma_start(
                    xbk[:], din["xtb"].ap()[k:k + 1, 0:NP].to_broadcast((128, NP)))
                xb.append(xbk)

            # ---------------- edge loop: NG groups of GS i-nodes ----------------
            # Software-pipelined: group g+1's m1pre matmuls and B''-add are
            # emitted before group g's we2 stage so the PE (strict FIFO) can
            # fill the next group's PSUM while ACT works on the current one.
            ps1_t = [None] * (NG + 1)
            m1pre_t = [None] * (NG + 1)

            def emit_m1pre(g):
                ps1 = ps.tile([128, 3, 512], F32, tag="ps", name=f"ps1_{g}")
                for b in range(3):
                    nc.tensor.matmul(
                        ps1[:, b, 0:BW], aTc[g // 4][0:GS + 1, g % 4, :],
                        t10[0:GS + 1, g * GW + b * BW: g * GW + (b + 1) * BW],
                        start=True, stop=True)
                m1pre = lp.tile([128, GW], BF16, tag="m1pre", name=f"m1pre_{g}")
                nc.vector.tensor_tensor(
                    out=m1pre[:].rearrange("p (b t j) -> p b t j", b=3, j=NP),
                    in0=ps1[:, :, 0:BW].rearrange("p b (t j) -> p b t j", j=NP),
                    in1=bpp[:].rearrange("p (o u j) -> p o u j", o=1, u=1)
                        .to_broadcast((128, 3, 3, NP)),
                    op=OP.add)
                ps1_t[g] = ps1
                m1pre_t[g] = m1pre

            # node-path halves: everything except the final Sqrt-dependent
            # normalization, so half A can run while the loop finishes
            n1sb = {}
            h2t = {}
            mv = {}

            def emit_node_half(half):
                lo = 0 if half == "A" else 128
                n1X = n1A if half == "A" else n1B
                n1s = sb.tile([H, 128], BF16, tag=f"n1sb{half}",
                              name=f"n1sb{half}")
                nc.scalar.activation(out=n1s[:], in_=n1X[:, :], func=SILU,
                                     bias=t["bn1"][:])
                if not use_silu:
                    n1p = sb.tile([H, 128], BF16, tag=f"n1p{half}",
                                  name=f"n1p{half}")
                    nc.vector.tensor_copy(n1p[:], n1X[:, :])
                    nc.vector.tensor_tensor(out=n1s[:], in0=n1s[:], in1=n1p[:],
                                            op=OP.mult)
                h2ps = ps.tile([D, 512], F32, tag="ps", name=f"h2ps{half}")
                nc.tensor.matmul(h2ps[:, 0:128], t["wn2"][:], n1s[:],
                                 start=True, stop=True)
                h2X = sb.tile([D, 128], F32, tag=f"h2{half}", name=f"h2{half}")
                nc.vector.tensor_scalar(out=h2X[:], in0=h2ps[:, 0:128],
                                        scalar1=t["bn2"][:], scalar2=None,
                                        op0=OP.add)
                nc.vector.tensor_tensor(out=h2X[:], in0=h2X[:],
                                        in1=t["hT32"][:, lo:lo + 128], op=OP.add)
                trp = ps.tile([128, 512], F32, tag="ps", name=f"trp{half}")
                nc.tensor.transpose(trp[:, 0:128], h2X[:], t["identf"][:])
                h2tX = sb.tile([128, D], F32, tag=f"h2t{half}", name=f"h2t{half}")
                nc.vector.tensor_copy(h2tX[:], trp[:, 0:128])
                st = sb.tile([128, 6], F32, tag=f"st{half}", name=f"st{half}")
                mvX = sb.tile([128, 2], F32, tag=f"mv{half}", name=f"mv{half}")
                nc.vector.bn_stats(out=st[:], in_=h2tX[:])
                nc.vector.bn_aggr(out=mvX[:], in_=st[:])
                n1sb[half] = n1s
                h2t[half] = h2tX
                mv[half] = mvX

            emit_m1pre(0)
            for g in range(NG):
                i0 = g * GS
                if g + 1 < NG:
                    emit_m1pre(g + 1)
                m1pre = m1pre_t[g]
                # SiLU -> m1
                m1 = lp.tile([128, GW], BF16, tag="m1")
                nc.scalar.activation(out=m1[:], in_=m1pre[:], func=SILU,
                                     bias=t["be1"][:])
                if not use_silu:
                    nc.vector.tensor_tensor(out=m1[:], in0=m1[:], in1=m1pre[:],
                                            op=OP.mult)
                # m2 = SiLU(we2^T m1 + be2); ps2 is reused for c1 and W below
                ps2 = ps.tile([128, 3, 512], F32, tag="ps", name=f"ps2_{g}")
                for b in range(3):
                    nc.tensor.matmul(ps2[:, b, 0:BW], t["we2"][:],
                                     m1[:, b * BW:(b + 1) * BW],
                                     start=True, stop=True)
                m2 = lp.tile([128, GW], BF16, tag="m2")
                nc.scalar.activation(
                    out=m2[:].rearrange("p (b w) -> p b w", b=3),
                    in_=ps2[:, :, 0:BW], func=SILU, bias=t["be2"][:])
                if not use_silu:
                    m2p = lp.tile([128, GW], BF16, tag="m2p")
                    nc.vector.tensor_copy(
                        m2p[:].rearrange("p (b w) -> p b w", b=3),
                        ps2[:, :, 0:BW])
                    nc.vector.tensor_tensor(out=m2[:], in0=m2[:], in1=m2p[:],
                                            op=OP.mult)
                # agg += per-i row sums; fold into the node-MLP psum now
                nc.vector.reduce_sum(
                    agg[:, i0:i0 + GS],
                    m2[:].rearrange("p (t j) -> p t j", j=NP),
                    axis=mybir.AxisListType.X)
                nc.vector.tensor_copy(aggb[:, i0:i0 + GS], agg[:, i0:i0 + GS])
                for tl, lo, hi, base in (
                        (n1A, i0, min(i0 + GS, 128), 0),
                        (n1B, max(i0, 128), i0 + GS, 128)):
                    if hi > lo:
                        nc.tensor.matmul(
                            tl[:, lo - base:hi - base], t["wn1b"][:],
                            aggb[:, lo:hi], start=False,
                            stop=(hi == 128 or g == NG - 1))
                if i0 < 128 <= i0 + GS:
                    emit_node_half("A")
                # c1 = SiLU(wc1^T m2 + bc1), reusing ps2
                for b in range(3):
                    nc.tensor.matmul(ps2[:, b, 0:BW], t["wc1"][:],
                                     m2[:, b * BW:(b + 1) * BW],
                                     start=True, stop=True)
                c1 = lp.tile([128, GW], BF16, tag="c1")
                nc.scalar.activation(
                    out=c1[:].rearrange("p (b w) -> p b w", b=3),
                    in_=ps2[:, :, 0:BW], func=SILU, bias=t["bc1"][:])
                if not use_silu:
                    c1p = lp.tile([128, GW], BF16, tag="c1p")
                    nc.vector.tensor_copy(
                        c1p[:].rearrange("p (b w) -> p b w", b=3),
                        ps2[:, :, 0:BW])
                    nc.vector.tensor_tensor(out=c1[:], in0=c1[:], in1=c1p[:],
                                            op=OP.mult)
                # W row strips into ps2's partition-0 rows
                for b in range(3):
                    nc.tensor.matmul(ps2[0:1, b, 0:BW], t["wc2"][:],
                                     c1[:, b * BW:(b + 1) * BW],
                                     start=True, stop=True)
                wst = lp.tile([1, GW], BF16, tag="wst")
                nc.scalar.activation(
                    out=wst[:].rearrange("p (b w) -> p b w", b=3),
                    in_=ps2[0:1, :, 0:BW], func=AF.Tanh)
                # repack the strip into row-major W (partition-crossing DMA)
                if i0 + GS <= 128:
                    nc.sync.dma_start(wrow0[i0:i0 + GS, :], wst[:])
                elif i0 >= 128:
                    nc.sync.dma_start(wrow1[i0 - 128:i0 - 128 + GS, :], wst[:])
                else:
                    n0 = 128 - i0
                    nc.sync.dma_start(wrow0[i0:128, :], wst[0:1, 0:n0 * NP])
                    nc.sync.dma_start(wrow1[0:GS - n0, :], wst[0:1, n0 * NP:])

            # ---------------- coord path ----------------
            for blk, rows in ((0, 128), (1, R2)):
                wr = wrow0 if blk == 0 else wrow1
                sco = lp.tile([128, 1], F32, tag="sco")
                nc.vector.reduce_sum(sco[0:rows, :], wr[0:rows, :],
                                     axis=mybir.AxisListType.X)
                tco = lp.tile([128, 3], F32, tag="tco")
                scr = lp.tile([128, NP], BF16, tag="scr")
                for k in range(3):
                    nc.vector.tensor_tensor(out=scr[0:rows, :], in0=wr[0:rows, :],
                                            in1=xb[k][0:rows, :], op=OP.mult)
                    nc.vector.reduce_sum(tco[0:rows, k:k + 1], scr[0:rows, :],
                                         axis=mybir.AxisListType.X)
                # x_out = x + (x * S - T) * 0.1/N
                u1 = lp.tile([128, 3], F32, tag="u1")
                nc.vector.tensor_scalar(out=u1[0:rows, :], in0=t["x32"][0:rows, blk, :],
                                        scalar1=sco[0:rows, :], scalar2=CSCALE,
                                        op0=OP.mult, op1=OP.mult)
                u2 = lp.tile([128, 3], F32, tag="u2")
                nc.vector.tensor_scalar_mul(out=u2[0:rows, :], in0=tco[0:rows, 0:3],
                                            scalar1=CSCALE)
                nc.vector.tensor_tensor(out=u1[0:rows, :], in0=u1[0:rows, :],
                                        in1=u2[0:rows, :], op=OP.subtract)
                nc.vector.tensor_tensor(out=u1[0:rows, :], in0=u1[0:rows, :],
                                        in1=t["x32"][0:rows, blk, :], op=OP.add)
                nc.sync.dma_start(xo_d.ap()[blk * 128: blk * 128 + rows, :],
                                  u1[0:rows, :])
            # untouched nodes: x_out = x
            nc.sync.dma_start(xo_d.ap()[NP:N, :], t["x32"][R2:128, 1, :])

            # ---------------- node path (half B) + LayerNorm finals ----------------
            emit_node_half("B")
            epsv = sb.tile([128, 1], F32, tag="epsv")
            nc.vector.memset(epsv[:], LN_EPS)
            for half in ("A", "B"):
                lo = 0 if half == "A" else 128
                sd = lp.tile([128, 1], F32, tag="sd")
                nc.scalar.activation(out=sd[:], in_=mv[half][:, 1:2],
                                     func=AF.Sqrt, bias=epsv[:])
                rstd = lp.tile([128, 1], F32, tag="rstd")
                nc.vector.reciprocal(rstd[:], sd[:])
                ho = lp.tile([128, D], F32, tag="ho")
                nc.vector.tensor_scalar(out=ho[:], in0=h2t[half][:],
                                        scalar1=mv[half][:, 0:1], scalar2=rstd[:],
                                        op0=OP.subtract, op1=OP.mult)
                nc.vector.tensor_tensor(out=ho[:], in0=ho[:], in1=gamB[:],
                                        op=OP.mult)
                nc.vector.tensor_tensor(out=ho[:], in0=ho[:], in1=betB[:],
                                        op=OP.add)
                nc.sync.dma_start(ho_d.ap()[lo:lo + 128, :], ho[:])

    nc.compile()
    return nc


def host_prep(inputs):
    """Permute/compact/cast the inputs into 8 per-core input maps."""
    h = np.asarray(inputs["h"], np.float32)
    x = np.asarray(inputs["x"], np.float32)
    mask = np.asarray(inputs["mask"])
    we1 = np.asarray(inputs["we1"], np.float32)

    perms = []
    shared = {
        "we1a": we1[:D].astype(BF),
        "we1b": we1[D:2 * D].astype(BF),
        "cw2": np.stack([we1[2 * D], np.ones(H, np.float32)]).astype(BF),
        "we2": np.asarray(inputs["we2"], np.float32).astype(BF),
        "wc1": np.asarray(inputs["wc1"], np.float32).astype(BF),
        "wc2": np.asarray(inputs["wc2"], np.float32).astype(BF),
        "wn1a": np.asarray(inputs["wn1"], np.float32)[:D].astype(BF),
        "wn1b": np.asarray(inputs["wn1"], np.float32)[D:].astype(BF),
        "wn2": np.asarray(inputs["wn2"], np.float32).astype(BF),
        "be1": np.asarray(inputs["be1"], np.float32).reshape(H, 1),
        "be2": np.asarray(inputs["be2"], np.float32).reshape(H, 1),
        "bc1": np.asarray(inputs["bc1"], np.float32).reshape(H, 1),
        "bn1": np.asarray(inputs["bn1"], np.float32).reshape(H, 1),
        "bn2": np.asarray(inputs["bn2"], np.float32).reshape(D, 1),
        "gamma": np.asarray(inputs["gamma"], np.float32).reshape(1, D),
        "beta": np.asarray(inputs["beta"], np.float32).reshape(1, D),
        "identf": np.eye(128, dtype=np.float32),
    }
    ind9 = np.zeros((GS, GS, NP), np.float32)
    for k in range(GS):
        ind9[k, k, :] = 1.0
    shared["ind9"] = ind9.reshape(GS, GW).astype(BF)

    in_maps = []
    for b in range(B):
        mb = mask[b].astype(bool)
        perm = np.concatenate([np.nonzero(mb)[0], np.nonzero(~mb)[0]])
        perms.append(perm)
        hp = h[b][perm]                      # [N, D]
        xp = x[b][perm]                      # [N, 3]
        mp = mb[perm].astype(np.float32)     # [N]
        core = dict(shared)
        core.update({
            "hT32": np.ascontiguousarray(hp.T),
            "hTb": np.ascontiguousarray(hp.T).astype(BF),
            "xt32": np.ascontiguousarray(xp.T),
            "xtb": np.ascontiguousarray(xp.T).astype(BF),
            "x32": np.ascontiguousarray(xp.reshape(2, 128, 3).transpose(1, 0, 2)),
            "mnegb": (MASK_NEG * (1.0 - mp)).reshape(1, N).astype(BF),
        })
        in_maps.append(core)
    return in_maps, perms


def host_post(results, perms):
    h_out = np.empty((B, N, D), np.float32)
    x_out = np.empty((B, N, 3), np.float32)
    for b in range(B):
        inv = np.empty(N, np.int64)
        inv[perms[b]] = np.arange(N)
        h_out[b] = results[b]["h_out"][inv]
        x_out[b] = results[b]["x_out"][inv]
    return h_out, x_out


_CACHED_NC = None


def kernel(**inputs):
    mask = np.asarray(inputs["mask"])
    fast_ok = (
        mask.shape == (B, N)
        and np.asarray(inputs["h"]).shape == (B, N, D)
        and mask.sum(axis=1).max() <= NP
        and all(np.abs(np.asarray(inputs[k])).max() == 0.0
                for k in ("be2", "bc1"))
    )
    if not fast_ok:
        return _numpy_reference(**{k: np.asarray(v) for k, v in inputs.items()})

    global _CACHED_NC
    if _CACHED_NC is None:
        _CACHED_NC = build_nc(use_silu=True)
    nc = _CACHED_NC

    from concourse import bass_utils
    in_maps, perms = host_prep(inputs)
    res = bass_utils.run_bass_kernel_spmd(nc, in_maps, core_ids=list(range(B)))
    return host_post(res.results, perms)


if __name__ == "__main__":
    print("kernel.py is a library; see test.py")


# revision 28
# speedup vs baseline: 1.0200x; 1.0200x over previous
"""Trainium2 Bass kernel for nn_E_GCL (EGNN graph conv layer).

Contract: kernel(**inputs) takes the FULL unsharded inputs (as produced by
setup_inputs) and returns the full outputs (h_out [B,N,D], x_out [B,N,3]).

Strategy (hardcoded for B=8, N=256, D=H=128, 8 NeuronCores):
- Data-parallel over batch: core b processes sample b.
- Host-side: per sample, permute nodes so mask==1 nodes come first, and
  restrict the O(N^2) edge computation to the first NP=153 nodes (actual
  active counts are <= 147; guarded by a numpy fallback). Masked nodes
  inside [0,NP) are neutralized by adding -1e4 to the edge-MLP layer-1
  pre-activation (SiLU(-1e4)=0 exactly, and zero biases make all
  downstream masked quantities exactly 0, matching the reference's cm
  masking).
- Device: all big matmuls in bf16 with fp32 PSUM accumulation.
  dist_sq is decomposed as r_i + r_j - 2 x_i.x_j; the r_i/r_j rank-1
  terms are folded into per-node A''/B'' vectors, so the per-edge
  layer-1 pre-activation is  A''_i + B''_j + we1_c (x) (-2 x_i.x_j),
  assembled directly in PSUM by one fused K=10 matmul per bank (indicator
  rows for A'' + a Db row for the dist channel), with B'' added by one
  broadcast DVE op per group.
- Edge loop: 17 groups of 9 i-nodes; per group three [128, 1377]
  activation passes (SiLU m1, SiLU m2, SiLU c1). W = tanh(c1 @ wc2) is
  computed as [1, GW] row strips (stationary wc2), tanh'd on ACT, then
  DMA-repacked to row-major W; coord sums are DVE multiply-reduces
  against partition-broadcast x columns.
- Node path (all 256 nodes): agg_m from per-group reduces; node MLP,
  residual in fp32, LayerNorm via PE transpose + bn_stats.
"""

import os
import sys

import numpy as np

if "/opt/trn_rl_repo" not in sys.path:
    sys.path.insert(0, "/opt/trn_rl_repo")

import ml_dtypes

BF = ml_dtypes.bfloat16

# problem constants
B, N, D, H = 8, 256, 128, 128
NP = 153          # padded compacted node count for the edge computation
GS = 9            # i-nodes per group
NG = NP // GS     # 17 groups
BW = 3 * NP       # per-PSUM-bank used width (459 fp32 <= 512)
R2 = NP - 128     # rows in the second partition block (25)
GW = GS * NP      # per-group width (1296)
LN_EPS = 1e-5
MASK_NEG = -1.0e4
CSCALE = 0.1 / N  # tanh post-scale folded with the 1/N coord normalizer


def _silu(v):
    return v * (1.0 / (1.0 + np.exp(-v)))


def _numpy_reference(h, x, mask, we1, be1, we2, be2, wn1, bn1, wn2, bn2,
                     wc1, bc1, wc2, gamma, beta):
    """Exact fp32 fallback (only used if the fast-path guards fail)."""
    m = mask.astype(h.dtype)
    cm = m[:, :, None] * m[:, None, :]
    hi = np.broadcast_to(h[:, :, None, :], (B, N, N, D))
    hj = np.broadcast_to(h[:, None, :, :], (B, N, N, D))
    rel_x = x[:, :, None, :] - x[:, None, :, :]
    dist_sq = np.sum(rel_x * rel_x, axis=-1, keepdims=True) + 1e-8
    edge_input = np.concatenate([hi, hj, np.broadcast_to(dist_sq, (B, N, N, 1))], axis=-1)
    m_ij = _silu(_silu(edge_input @ we1 + be1) @ we2 + be2)
    m_ij = m_ij * cm[..., None]
    w_ij = np.tanh(_silu(m_ij @ wc1 + bc1) @ wc2) * 0.1
    w_ij = w_ij * cm[..., None]
    coord_diff = np.sum(rel_x * w_ij, axis=2) / N
    x_out = x + coord_diff
    agg_m = np.sum(m_ij, axis=2)
    h_new = _silu(np.concatenate([h, agg_m], axis=-1) @ wn1 + bn1) @ wn2 + bn2
    h2 = h + h_new
    mu = h2.mean(axis=-1, keepdims=True)
    var = ((h2 - mu) ** 2).mean(axis=-1, keepdims=True)
    h_out = (h2 - mu) / np.sqrt(var + LN_EPS) * gamma + beta
    return h_out.astype(np.float32), x_out.astype(np.float32)


def build_nc(use_silu=True):
    """Build the single-core Bass program (same NEFF runs SPMD on 8 cores)."""
    import concourse.bacc as bacc
    import concourse.tile as tile
    import concourse.mybir as mybir

    F32 = mybir.dt.float32
    BF16 = mybir.dt.bfloat16
    AF = mybir.ActivationFunctionType
    OP = mybir.AluOpType

    nc = bacc.Bacc("TRN2", target_bir_lowering=False, debug=False)

    # ---- DRAM tensors: shared weights (same values on every core) ----
    din = {}

    def dram_in(name, shape, dt):
        din[name] = nc.dram_tensor(name, list(shape), dt, kind="ExternalInput")
        return din[name]

    dram_in("we1a", (D, H), BF16)       # we1[:D]
    dram_in("we1b", (D, H), BF16)       # we1[D:2D]
    dram_in("cw2", (2, H), BF16)        # [we1_c ; ones]
    dram_in("we2", (H, H), BF16)
    dram_in("wc1", (H, H), BF16)
    dram_in("wc2", (H, 1), BF16)
    dram_in("wn1a", (D, H), BF16)       # wn1[:D]
    dram_in("wn1b", (H, H), BF16)       # wn1[D:]
    dram_in("wn2", (H, D), BF16)
    dram_in("be1", (H, 1), F32)
    dram_in("be2", (H, 1), F32)
    dram_in("bc1", (H, 1), F32)
    dram_in("bn1", (H, 1), F32)
    dram_in("bn2", (D, 1), F32)
    dram_in("gamma", (1, D), F32)
    dram_in("beta", (1, D), F32)
    dram_in("identf", (128, 128), F32)
    dram_in("ind9", (GS, GW), BF16)     # IND9[k, (k', j)] = (k == k')
    # ---- per-core (per-sample, permuted) tensors ----
    dram_in("hT32", (D, N), F32)        # h permuted, transposed, fp32
    dram_in("hTb", (D, N), BF16)
    dram_in("xt32", (3, N), F32)        # x permuted, transposed
    dram_in("xtb", (3, N), BF16)
    dram_in("x32", (128, 2, 3), F32)    # x permuted, natural, [p, blk, 3]
    dram_in("mnegb", (1, N), BF16)      # -1e4 * (1 - mask), permuted

    ho_d = nc.dram_tensor("h_out", [N, D], F32, kind="ExternalOutput")
    xo_d = nc.dram_tensor("x_out", [N, 3], F32, kind="ExternalOutput")

    SILU = AF.Silu if use_silu else AF.Sigmoid

    with tile.TileContext(nc) as tc:
        with (
            tc.tile_pool(name="sb", bufs=1) as sb,          # persistent sbuf
            tc.tile_pool(name="lp", bufs=3) as lp,          # loop sbuf tiles
            tc.tile_pool(name="ps", bufs=2, space="PSUM") as ps,
            tc.tile_pool(name="nps", bufs=1, space="PSUM") as nps,
        ):
            # ---------------- load everything into SBUF ----------------
            t = {}
            for name, shape, dt in [
                ("hTb", (D, N), BF16), ("xtb", (3, N), BF16),
                ("xt32", (3, N), F32), ("hT32", (D, N), F32),
                ("x32", (128, 2, 3), F32),
                ("we1a", (D, H), BF16), ("we1b", (D, H), BF16),
                ("cw2", (2, H), BF16), ("we2", (H, H), BF16),
                ("wc1", (H, H), BF16), ("wc2", (H, 1), BF16),
                ("wn1a", (D, H), BF16), ("wn1b", (H, H), BF16),
                ("wn2", (H, D), BF16),
                ("be1", (H, 1), F32), ("be2", (H, 1), F32),
                ("bc1", (H, 1), F32), ("bn1", (H, 1), F32),
                ("bn2", (D, 1), F32),
                ("identf", (128, 128), F32), ("ind9", (GS, GW), BF16),
            ]:
                t[name] = sb.tile(list(shape), dt, tag=f"t_{name}",
                                  name=f"t_{name}")
                eng = nc.gpsimd if name[0] == "w" or name[0] == "b" else nc.sync
                eng.dma_start(t[name][:], din[name].ap())

            # gamma/beta broadcast to [128, D] once
            gamB = sb.tile([128, D], F32, tag="gamB")
            betB = sb.tile([128, D], F32, tag="betB")
            nc.scalar.dma_start(gamB[:], din["gamma"].ap().to_broadcast((128, D)))
            nc.scalar.dma_start(betB[:], din["beta"].ap().to_broadcast((128, D)))

            # ---------------- per-sample prep ----------------
            # r = |x|^2 row:  Square(xt) then ones3^T @ xt2
            xt2 = sb.tile([3, N], BF16, tag="xt2")
            nc.scalar.activation(out=xt2[:], in_=t["xt32"][:], func=AF.Square)
            ones3 = sb.tile([3, 1], BF16, tag="ones3")
            nc.vector.memset(ones3[:], 1.0)
            r_ps = ps.tile([1, 512], F32, tag="ps")
            nc.tensor.matmul(r_ps[0:1, 0:N], ones3[:], xt2[:], start=True, stop=True)
            # rm2 = [r ; mneg] (2, N) bf16
            rm2 = sb.tile([2, N], BF16, tag="rm2")
            nc.vector.tensor_copy(rm2[0:1, :], r_ps[0:1, 0:N])
            nc.sync.dma_start(rm2[1:2, :], din["mnegb"].ap())

            # B'' = we1b^T hT + we1c (x) r + ones (x) mneg   -> bf16 [H, NP]
            bpp_ps = ps.tile([H, 512], F32, tag="ps")
            nc.tensor.matmul(bpp_ps[:, 0:N], t["we1b"][:], t["hTb"][:],
                             start=True, stop=False)
            nc.tensor.matmul(bpp_ps[:, 0:N], t["cw2"][:], rm2[:],
                             start=False, stop=True)
            bpp = sb.tile([H, NP], BF16, tag="bpp")
            nc.vector.tensor_copy(bpp[:], bpp_ps[:, 0:NP])

            # A''^T chunk tiles: aTc[c][k, u, h] for g = 4c+u; row GS = we1_c
            # so a single K=GS+1 matmul applies A'' and the dist channel
            NCH = (NG + 3) // 4
            aTc = []
            for c in range(NCH):
                gs_in = min(4, NG - 4 * c)
                ac = sb.tile([GS + 1, gs_in, H], BF16, tag=f"aTc{c}",
                             name=f"aTc{c}")
                pa = ps.tile([GS, 4, H], F32, tag="ps", name=f"pa_{c}")
                for u in range(gs_in):
                    g = 4 * c + u
                    nc.tensor.matmul(pa[0:GS, u, :],
                                     t["hTb"][:, g * GS:(g + 1) * GS],
                                     t["we1a"][:], start=True, stop=False)
                    nc.tensor.matmul(pa[0:GS, u, :],
                                     rm2[:, g * GS:(g + 1) * GS],
                                     t["cw2"][:], start=False, stop=True)
                nc.vector.tensor_copy(ac[0:GS, :, :], pa[0:GS, 0:gs_in, :])
                nc.sync.dma_start(
                    ac[GS:GS + 1, :, :],
                    din["cw2"].ap()[0:1, :].rearrange("p (o h) -> p o h", o=1)
                        .to_broadcast((1, gs_in, H)))
                aTc.append(ac)

            # T10: rhs rows for the fused m1pre matmul.  Rows 0..GS-1 hold the
            # IND indicator pattern (replicated per group), row GS holds
            # Db[i, j] = -2 x_i . x_j flattened in (i, j) order.
            t10 = sb.tile([GS + 1, NG * GW], BF16, tag="t10")
            nc.scalar.dma_start(
                t10[0:GS, :],
                din["ind9"].ap().rearrange("k (o w) -> k o w", o=1)
                    .to_broadcast((GS, NG, GW)))
            db = sb.tile([128, 2, NP], BF16, tag="db")
            for blk, rows in ((0, 128), (1, R2)):
                gp = ps.tile([128, 512], F32, tag="ps")
                nc.tensor.matmul(gp[0:rows, 0:NP],
                                 t["xtb"][:, blk * 128: blk * 128 + rows],
                                 t["xtb"][:, 0:NP], start=True, stop=True)
                nc.vector.tensor_scalar(out=db[0:rows, blk, :], in0=gp[0:rows, 0:NP],
                                        scalar1=-2.0, scalar2=None, op0=OP.mult)
                nc.sync.dma_start(
                    t10[GS:GS + 1, blk * 128 * NP: blk * 128 * NP + rows * NP],
                    db[0:rows, blk, :])

            # AGG (sum_j m_ij) fp32 [H, N]; cols >= NP stay 0
            agg = sb.tile([H, N], F32, tag="agg")
            nc.vector.memset(agg[:], 0.0)
            aggb = sb.tile([H, NP], BF16, tag="aggb")

            # node-MLP layer-1 psum accumulates across the edge loop, split
            # into column halves so the first half's LN can overlap the loop
            n1A = nps.tile([H, 128], F32, tag="n1A")
            n1B = nps.tile([H, 128], F32, tag="n1B")
            nc.tensor.matmul(n1A[:, :], t["wn1a"][:], t["hTb"][:, 0:128],
                             start=True, stop=False)
            nc.tensor.matmul(n1B[:, :], t["wn1a"][:], t["hTb"][:, 128:N],
                             start=True, stop=False)

            # row-major W accumulators (filled by per-group DMA repacks).
            # wrow* holds tanh'd rows (ACT-crossed groups); wrp* holds raw
            # rows (DVE-crossed groups, tanh applied after the loop).
            wrow0 = sb.tile([128, NP], BF16, tag="wrow0")
            wrow1 = sb.tile([R2, NP], BF16, tag="wrow1")
            wrp0 = sb.tile([128, NP], BF16, tag="wrp0")
            wrp1 = sb.tile([R2, NP], BF16, tag="wrp1")
            for z in (wrow0, wrow1, wrp0, wrp1):
                nc.vector.memset(z[:], 0.0)
            # partition-broadcast x columns for the coord reduces
            xb = []
            for k in range(3):
                xbk = sb.tile([128, NP], BF16, tag=f"xb{k}", name=f"xb{k}")
                nc.scalar.dma_start(
                    xbk[:], din["xtb"].ap()[k:k + 1, 0:NP].to_broadcast((128, NP)))
                xb.append(xbk)

            # ---------------- edge loop: NG groups of GS i-nodes ----------------
            # Software-pipelined: group g+1's m1pre matmuls and B''-add are
            # emitted before group g's we2 stage so the PE (strict FIFO) can
            # fill the next group's PSUM while ACT works on the current one.
            ps1_t = [None] * (NG + 1)
            m1pre_t = [None] * (NG + 1)

            def emit_m1pre(g):
                ps1 = ps.tile([128, 3, 512], F32, tag="ps", name=f"ps1_{g}")
                for b in range(3):
                    nc.tensor.matmul(
                        ps1[:, b, 0:BW], aTc[g // 4][0:GS + 1, g % 4, :],
                        t10[0:GS + 1, g * GW + b * BW: g * GW + (b + 1) * BW],
                        start=True, stop=True)
                m1pre = lp.tile([128, GW], BF16, tag="m1pre", name=f"m1pre_{g}")
                nc.vector.tensor_tensor(
                    out=m1pre[:].rearrange("p (b t j) -> p b t j", b=3, j=NP),
                    in0=ps1[:, :, 0:BW].rearrange("p b (t j) -> p b t j", j=NP),
                    in1=bpp[:].rearrange("p (o u j) -> p o u j", o=1, u=1)
                        .to_broadcast((128, 3, 3, NP)),
                    op=OP.add)
                ps1_t[g] = ps1
                m1pre_t[g] = m1pre

            # node-path halves: everything except the final Sqrt-dependent
            # normalization, so half A can run while the loop finishes
            n1sb = {}
            h2t = {}
            mv = {}

            def emit_node_half(half):
                lo = 0 if half == "A" else 128
                n1X = n1A if half == "A" else n1B
                n1s = sb.tile([H, 128], BF16, tag=f"n1sb{half}",
                              name=f"n1sb{half}")
                nc.scalar.activation(out=n1s[:], in_=n1X[:, :], func=SILU,
                                     bias=t["bn1"][:])
                if not use_silu:
                    n1p = sb.tile([H, 128], BF16, tag=f"n1p{half}",
                                  name=f"n1p{half}")
                    nc.vector.tensor_copy(n1p[:], n1X[:, :])
                    nc.vector.tensor_tensor(out=n1s[:], in0=n1s[:], in1=n1p[:],
                                            op=OP.mult)
                h2ps = ps.tile([D, 512], F32, tag="ps", name=f"h2ps{half}")
                nc.tensor.matmul(h2ps[:, 0:128], t["wn2"][:], n1s[:],
                                 start=True, stop=True)
                h2X = sb.tile([D, 128], F32, tag=f"h2{half}", name=f"h2{half}")
                nc.vector.tensor_scalar(out=h2X[:], in0=h2ps[:, 0:128],
                                        scalar1=t["bn2"][:], scalar2=None,
                                        op0=OP.add)
                nc.vector.tensor_tensor(out=h2X[:], in0=h2X[:],
                                        in1=t["hT32"][:, lo:lo + 128], op=OP.add)
                trp = ps.tile([128, 512], F32, tag="ps", name=f"trp{half}")
                nc.tensor.transpose(trp[:, 0:128], h2X[:], t["identf"][:])
                h2tX = sb.tile([128, D], F32, tag=f"h2t{half}", name=f"h2t{half}")
                nc.vector.tensor_copy(h2tX[:], trp[:, 0:128])
                st = sb.tile([128, 6], F32, tag=f"st{half}", name=f"st{half}")
                mvX = sb.tile([128, 2], F32, tag=f"mv{half}", name=f"mv{half}")
                nc.vector.bn_stats(out=st[:], in_=h2tX[:])
                nc.vector.bn_aggr(out=mvX[:], in_=st[:])
                n1sb[half] = n1s
                h2t[half] = h2tX
                mv[half] = mvX

            emit_m1pre(0)
            for g in range(NG):
                i0 = g * GS
                if g + 1 < NG:
                    emit_m1pre(g + 1)
                m1pre = m1pre_t[g]
                # SiLU -> m1
                m1 = lp.tile([128, GW], BF16, tag="m1")
                nc.scalar.activation(out=m1[:], in_=m1pre[:], func=SILU,
                                     bias=t["be1"][:])
                if not use_silu:
                    nc.vector.tensor_tensor(out=m1[:], in0=m1[:], in1=m1pre[:],
                                            op=OP.mult)
                # m2 = SiLU(we2^T m1 + be2); ps2 is reused for c1 and W below
                ps2 = ps.tile([128, 3, 512], F32, tag="ps", name=f"ps2_{g}")
                for b in range(3):
                    nc.tensor.matmul(ps2[:, b, 0:BW], t["we2"][:],
                                     m1[:, b * BW:(b + 1) * BW],
                                     start=True, stop=True)
                m2 = lp.tile([128, GW], BF16, tag="m2")
                nc.scalar.activation(
                    out=m2[:].rearrange("p (b w) -> p b w", b=3),
                    in_=ps2[:, :, 0:BW], func=SILU, bias=t["be2"][:])
                if not use_silu:
                    m2p = lp.tile([128, GW], BF16, tag="m2p")
                    nc.vector.tensor_copy(
                        m2p[:].rearrange("p (b w) -> p b w", b=3),
                        ps2[:, :, 0:BW])
                    nc.vector.tensor_tensor(out=m2[:], in0=m2[:], in1=m2p[:],
                                            op=OP.mult)
                # agg += per-i row sums; fold into the node-MLP psum now
                nc.vector.reduce_sum(
                    agg[:, i0:i0 + GS],
                    m2[:].rearrange("p (t j) -> p t j", j=NP),
                    axis=mybir.AxisListType.X)
                nc.vector.tensor_copy(aggb[:, i0:i0 + GS], agg[:, i0:i0 + GS])
                for tl, lo, hi, base in (
                        (n1A, i0, min(i0 + GS, 128), 0),
                        (n1B, max(i0, 128), i0 + GS, 128)):
                    if hi > lo:
                        nc.tensor.matmul(
                            tl[:, lo - base:hi - base], t["wn1b"][:],
                            aggb[:, lo:hi], start=False,
                            stop=(hi == 128 or g == NG - 1))
                if i0 < 128 <= i0 + GS:
                    emit_node_half("A")
                # c1 = SiLU(wc1^T m2 + bc1), reusing ps2
                for b in range(3):
                    nc.tensor.matmul(ps2[:, b, 0:BW], t["wc1"][:],
                                     m2[:, b * BW:(b + 1) * BW],
                                     start=True, stop=True)
                c1 = lp.tile([128, GW], BF16, tag="c1")
                nc.scalar.activation(
                    out=c1[:].rearrange("p (b w) -> p b w", b=3),
                    in_=ps2[:, :, 0:BW], func=SILU, bias=t["bc1"][:])
                if not use_silu:
                    c1p = lp.tile([128, GW], BF16, tag="c1p")
                    nc.vector.tensor_copy(
                        c1p[:].rearrange("p (b w) -> p b w", b=3),
                        ps2[:, :, 0:BW])
                    nc.vector.tensor_tensor(out=c1[:], in0=c1[:], in1=c1p[:],
                                            op=OP.mult)
                # W row strips into ps2's partition-0 rows
                for b in range(3):
                    nc.tensor.matmul(ps2[0:1, b, 0:BW], t["wc2"][:],
                                     c1[:, b * BW:(b + 1) * BW],
                                     start=True, stop=True)
                wst = lp.tile([1, GW], BF16, tag="wst")
                if g % 2 == 0:
                    nc.scalar.activation(
                        out=wst[:].rearrange("p (b w) -> p b w", b=3),
                        in_=ps2[0:1, :, 0:BW], func=AF.Tanh)
                    d0, d1 = wrow0, wrow1
                else:
                    nc.vector.tensor_copy(
                        wst[:].rearrange("p (b w) -> p b w", b=3),
                        ps2[0:1, :, 0:BW])
                    d0, d1 = wrp0, wrp1
                # repack the strip into row-major W (partition-crossing DMA)
                if i0 + GS <= 128:
                    nc.sync.dma_start(d0[i0:i0 + GS, :], wst[:])
                elif i0 >= 128:
                    nc.sync.dma_start(d1[i0 - 128:i0 - 128 + GS, :], wst[:])
                else:
                    n0 = 128 - i0
                    nc.sync.dma_start(d0[i0:128, :], wst[0:1, 0:n0 * NP])
                    nc.sync.dma_start(d1[0:GS - n0, :], wst[0:1, n0 * NP:])

            # ---------------- coord path ----------------
            # apply tanh to the DVE-crossed (raw) rows, then reduce both sets
            wq0 = sb.tile([128, NP], BF16, tag="wq0")
            wq1 = sb.tile([R2, NP], BF16, tag="wq1")
            nc.scalar.activation(out=wq0[:], in_=wrp0[:], func=AF.Tanh)
            nc.scalar.activation(out=wq1[:], in_=wrp1[:], func=AF.Tanh)
            for blk, rows in ((0, 128), (1, R2)):
                pair = (wrow0, wq0) if blk == 0 else (wrow1, wq1)
                sco = lp.tile([128, 2], F32, tag="sco")
                tco = lp.tile([128, 6], F32, tag="tco")
                for v, wr in enumerate(pair):
                    nc.vector.reduce_sum(sco[0:rows, v:v + 1], wr[0:rows, :],
                                         axis=mybir.AxisListType.X)
                    scr = lp.tile([128, NP], BF16, tag="scr")
                    for k in range(3):
                        nc.vector.tensor_tensor(out=scr[0:rows, :],
                                                in0=wr[0:rows, :],
                                                in1=xb[k][0:rows, :], op=OP.mult)
                        nc.vector.reduce_sum(tco[0:rows, 3 * v + k:3 * v + k + 1],
                                             scr[0:rows, :],
                                             axis=mybir.AxisListType.X)
                sc1 = lp.tile([128, 1], F32, tag="sc1")
                nc.vector.tensor_tensor(out=sc1[0:rows, :], in0=sco[0:rows, 0:1],
                                        in1=sco[0:rows, 1:2], op=OP.add)
                tc3 = lp.tile([128, 3], F32, tag="tc3")
                nc.vector.tensor_tensor(out=tc3[0:rows, :], in0=tco[0:rows, 0:3],
                                        in1=tco[0:rows, 3:6], op=OP.add)
                # x_out = x + (x * S - T) * 0.1/N
                u1 = lp.tile([128, 3], F32, tag="u1")
                nc.vector.tensor_scalar(out=u1[0:rows, :], in0=t["x32"][0:rows, blk, :],
                                        scalar1=sc1[0:rows, :], scalar2=CSCALE,
                                        op0=OP.mult, op1=OP.mult)
                u2 = lp.tile([128, 3], F32, tag="u2")
                nc.vector.tensor_scalar_mul(out=u2[0:rows, :], in0=tc3[0:rows, :],
                                            scalar1=CSCALE)
                nc.vector.tensor_tensor(out=u1[0:rows, :], in0=u1[0:rows, :],
                                        in1=u2[0:rows, :], op=OP.subtract)
                nc.vector.tensor_tensor(out=u1[0:rows, :], in0=u1[0:rows, :],
                                        in1=t["x32"][0:rows, blk, :], op=OP.add)
                nc.sync.dma_start(xo_d.ap()[blk * 128: blk * 128 + rows, :],
                                  u1[0:rows, :])
            # untouched nodes: x_out = x
            nc.sync.dma_start(xo_d.ap()[NP:N, :], t["x32"][R2:128, 1, :])

            # ---------------- node path (half B) + LayerNorm finals ----------------
            emit_node_half("B")
            epsv = sb.tile([128, 1], F32, tag="epsv")
            nc.vector.memset(epsv[:], LN_EPS)
            for half in ("A", "B"):
                lo = 0 if half == "A" else 128
                sd = lp.tile([128, 1], F32, tag="sd")
                nc.scalar.activation(out=sd[:], in_=mv[half][:, 1:2],
                                     func=AF.Sqrt, bias=epsv[:])
                rstd = lp.tile([128, 1], F32, tag="rstd")
                nc.vector.reciprocal(rstd[:], sd[:])
                ho = lp.tile([128, D], F32, tag="ho")
                nc.vector.tensor_scalar(out=ho[:], in0=h2t[half][:],
                                        scalar1=mv[half][:, 0:1], scalar2=rstd[:],
                                        op0=OP.subtract, op1=OP.mult)
                nc.vector.tensor_tensor(out=ho[:], in0=ho[:], in1=gamB[:],
                                        op=OP.mult)
                nc.vector.tensor_tensor(out=ho[:], in0=ho[:], in1=betB[:],
                                        op=OP.add)
                nc.sync.dma_start(ho_d.ap()[lo:lo + 128, :], ho[:])

    nc.compile()
    return nc


def host_prep(inputs):
    """Permute/compact/cast the inputs into 8 per-core input maps."""
    h = np.asarray(inputs["h"], np.float32)
    x = np.asarray(inputs["x"], np.float32)
    mask = np.asarray(inputs["mask"])
    we1 = np.asarray(inputs["we1"], np.float32)

    perms = []
    shared = {
        "we1a": we1[:D].astype(BF),
        "we1b": we1[D:2 * D].astype(BF),
        "cw2": np.stack([we1[2 * D], np.ones(H, np.float32)]).astype(BF),
        "we2": np.asarray(inputs["we2"], np.float32).astype(BF),
        "wc1": np.asarray(inputs["wc1"], np.float32).astype(BF),
        "wc2": np.asarray(inputs["wc2"], np.float32).astype(BF),
        "wn1a": np.asarray(inputs["wn1"], np.float32)[:D].astype(BF),
        "wn1b": np.asarray(inputs["wn1"], np.float32)[D:].astype(BF),
        "wn2": np.asarray(inputs["wn2"], np.float32).astype(BF),
        "be1": np.asarray(inputs["be1"], np.float32).reshape(H, 1),
        "be2": np.asarray(inputs["be2"], np.float32).reshape(H, 1),
        "bc1": np.asarray(inputs["bc1"], np.float32).reshape(H, 1),
        "bn1": np.asarray(inputs["bn1"], np.float32).reshape(H, 1),
        "bn2": np.asarray(inputs["bn2"], np.float32).reshape(D, 1),
        "gamma": np.asarray(inputs["gamma"], np.float32).reshape(1, D),
        "beta": np.asarray(inputs["beta"], np.float32).reshape(1, D),
        "identf": np.eye(128, dtype=np.float32),
    }
    ind9 = np.zeros((GS, GS, NP), np.float32)
    for k in range(GS):
        ind9[k, k, :] = 1.0
    shared["ind9"] = ind9.reshape(GS, GW).astype(BF)

    in_maps = []
    for b in range(B):
        mb = mask[b].astype(bool)
        perm = np.concatenate([np.nonzero(mb)[0], np.nonzero(~mb)[0]])
        perms.append(perm)
        hp = h[b][perm]                      # [N, D]
        xp = x[b][perm]                      # [N, 3]
        mp = mb[perm].astype(np.float32)     # [N]
        core = dict(shared)
        core.update({
            "hT32": np.ascontiguousarray(hp.T),
            "hTb": np.ascontiguousarray(hp.T).astype(BF),
            "xt32": np.ascontiguousarray(xp.T),
            "xtb": np.ascontiguousarray(xp.T).astype(BF),
            "x32": np.ascontiguousarray(xp.reshape(2, 128, 3).transpose(1, 0, 2)),
            "mnegb": (MASK_NEG * (1.0 - mp)).reshape(1, N).astype(BF),
        })
        in_maps.append(core)
    return in_maps, perms


def host_post(results, perms):
    h_out = np.empty((B, N, D), np.float32)
    x_out = np.empty((B, N, 3), np.float32)
    for b in range(B):
        inv = np.empty(N, np.int64)
        inv[perms[b]] = np.arange(N)
        h_out[b] = results[b]["h_out"][inv]
        x_out[b] = results[b]["x_out"][inv]
    return h_out, x_out


_CACHED_NC = None


def kernel(**inputs):
    mask = np.asarray(inputs["mask"])
    fast_ok = (
        mask.shape == (B, N)
        and np.asarray(inputs["h"]).shape == (B, N, D)
        and mask.sum(axis=1).max() <= NP
        and all(np.abs(np.asarray(inputs[k])).max() == 0.0
                for k in ("be2", "bc1"))
    )
    if not fast_ok:
        return _numpy_reference(**{k: np.asarray(v) for k, v in inputs.items()})

    global _CACHED_NC
    if _CACHED_NC is None:
        _CACHED_NC = build_nc(use_silu=True)
    nc = _CACHED_NC

    from concourse import bass_utils
    in_maps, perms = host_prep(inputs)
    res = bass_utils.run_bass_kernel_spmd(nc, in_maps, core_ids=list(range(B)))
    return host_post(res.results, perms)


if __name__ == "__main__":
    print("kernel.py is a library; see test.py")
